# revision 1
# baseline (speedup 1.0000x reference)
"""Contrastive patch loss (InfoNCE over sampled voxel patches) on 8 TRN2 NeuronCores.

Math
----
Reference computes, per patch p and batch b, cs[k,l] = <t2n[:,i_pk], t1n[:,i_pl]>
over k=512 sampled voxels (i = idx[p]), e = exp(cs/bw), then the mean over
(p,b,j) of -log(0.5*e_jj*(1/colsum_j + 1/rowsum_j) + eps).

Since every sampled voxel index lives in [0, 512), cs is just a gather of the
512x512 Gram matrix G_b = t2n^T @ t1n:  cs[k,l] = G_b[i_k, i_l].  With
E_b = exp(G_b/bw) and c_p[s] = multiplicity of voxel s in patch p:

    rowsum_j = (E_b @ c_p)[i_j]        colsum_j = (E_b^T @ c_p)[i_j]
    pos_j    = diag(E_b)[i_j]

and the sum over j of any f(i_j) equals sum_s c_p[s] f(s).  So the whole
(P,B,K,K) tensor never needs to exist:

    loss = -1/(P*B*K) * sum_{b,p,s} c_p[s] *
           log(0.5*diagE_b[s]*(1/CS_b[s,p] + 1/RS_b[s,p]) + eps)

where RS_b = E_b @ C^T and CS_b = E_b^T @ C^T are (512, P) matmuls.

Sharding: 8 cores = 2 batches x 4 patch-quarters.  Each core builds E for its
batch (normalize, Gram matmul, exp), then computes RS/CS/loss terms for its 32
patches.  Per-core partial sums are returned as a (128,4) tile; the host adds
the 8 partials and applies -1/(P*B*K).

Precision: matmul operands are bf16 (verified: final rel err ~2e-5 vs f64);
accumulation, exp/log and the loss reduction stay fp32.
"""

import math

import ml_dtypes
import numpy as np

import concourse.bacc as bacc
import concourse.tile as tile
from concourse import hw_specs, mybir
from concourse.bass_utils import run_bass_kernel_spmd
from concourse.masks import make_identity

# Pin every ACTIVATE to the one table set that holds ln+exp+square+copy, so
# the kernel pays a single ACT_TABLE_LOAD instead of ping-ponging between the
# per-function default sets.  Indices (act_func_set_id) are preserved; only
# membership of the other sets is hidden from the placement pass.
_PIN_SET = "natural_log_exp_and_others"
_orig_get_tables = hw_specs.get_activation_tables


def _pinned_tables(arch):
    tabs = _orig_get_tables(arch)
    return {k: (v if k == _PIN_SET else set()) for k, v in tabs.items()}


bacc.get_activation_tables = _pinned_tables

B, C, S = 2, 256, 512
P, K = 128, 512
BW = 0.05
EPS = 1e-5
N_CORES = 8
PQ = P // 4  # patches per core (32)
F32 = mybir.dt.float32
BF16 = mybir.dt.bfloat16
USE_VREC = True  # vector.reciprocal; False -> exp(-ln(x)) on ScalarE


def _build_program():
    nc = bacc.Bacc("TRN2", target_bir_lowering=False, debug=False, num_devices=N_CORES)

    feat2 = nc.dram_tensor("feat2", [C, S], BF16, kind="ExternalInput")
    feat1 = nc.dram_tensor("feat1", [C, S], BF16, kind="ExternalInput")
    cntp = nc.dram_tensor("cntp", [128, 128], BF16, kind="ExternalInput")
    partial = nc.dram_tensor("partial", [128, 4], F32, kind="ExternalOutput")

    with tile.TileContext(nc) as tc:
        with (
            tc.tile_pool(name="const", bufs=1) as const,
            tc.tile_pool(name="feat", bufs=1) as featp,
            tc.tile_pool(name="big", bufs=1) as big,
            tc.tile_pool(name="tmp", bufs=2) as tmp,
            tc.tile_pool(name="small", bufs=2) as small,
            tc.tile_pool(name="ps", bufs=2, space="PSUM") as ps,
            tc.tile_pool(name="ps_small", bufs=2, space="PSUM") as ps_small,
        ):
            ident_bf = const.tile([128, 128], BF16, name="ident_bf", tag="ident_bf")
            make_identity(nc, ident_bf)
            one_1x1 = const.tile([1, 1], F32, name="one11", tag="one11")
            nc.vector.memset(one_1x1, 1.0)
            ones_col_bf = const.tile([128, 1], BF16, name="ones_col_bf", tag="ocb")
            nc.vector.memset(ones_col_bf, 1.0)
            ones_row = const.tile([1, 128], BF16, name="ones_row", tag="ones_row")
            nc.vector.memset(ones_row, 1.0)
            eps_col = const.tile([128, 1], F32, name="eps_col", tag="eps_col")
            nc.vector.memset(eps_col, EPS)
            ln_ibw_col = const.tile([128, 1], F32, name="ln_ibw_col", tag="lbc")
            nc.vector.memset(ln_ibw_col, math.log(1.0 / BW))

            # One DMA per tensor: features as (128, 2, S) [q, c-chunk, s],
            # counts pre-packed on host as (128, 128) [q, 32t+p].
            f2t = featp.tile([128, 2, S], BF16, name="f2t", tag="f2t")
            f1t = featp.tile([128, 2, S], BF16, name="f1t", tag="f1t")
            nc.sync.dma_start(
                out=f2t, in_=feat2[:, :].rearrange("(i q) s -> q i s", q=128)
            )
            nc.sync.dma_start(
                out=f1t, in_=feat1[:, :].rearrange("(i q) s -> q i s", q=128)
            )
            f2b = [f2t[:, i, :] for i in range(2)]
            f1b = [f1t[:, i, :] for i in range(2)]
            cnt_all = const.tile([128, 128], BF16, name="cnt_all", tag="cnt_all")
            nc.sync.dma_start(out=cnt_all, in_=cntp[:, :])
            cnt = [cnt_all[:, PQ * t : PQ * (t + 1)] for t in range(4)]
            wcnt = cnt_all

            # --- channel-wise sumsq via ones-matmul on bf16 squares ---
            ss_ps = []
            for nm, fch in (("2", f2b), ("1", f1b)):
                ssp = ps_small.tile([1, S], F32, name=f"ss_ps{nm}", tag="sm_ps_a")
                for i in range(2):
                    sq = tmp.tile([128, S], BF16, name="sq", tag="sq")
                    nc.scalar.activation(
                        out=sq, in_=fch[i],
                        func=mybir.ActivationFunctionType.Square,
                    )
                    nc.tensor.matmul(
                        out=ssp, lhsT=ones_col_bf, rhs=sq,
                        start=(i == 0), stop=(i == 1),
                    )
                ss_ps.append(ssp)
            ss2_ps, ss1_ps = ss_ps

            # ln(ss); keep both Ln calls adjacent (one ACT table set visit)
            lns2 = small.tile([1, S], F32, name="lns2", tag="lns2")
            nc.scalar.activation(
                out=lns2, in_=ss2_ps, func=mybir.ActivationFunctionType.Ln
            )
            lns1 = small.tile([1, S], F32, name="lns1", tag="lns1")
            nc.scalar.activation(
                out=lns1, in_=ss1_ps, func=mybir.ActivationFunctionType.Ln
            )

            # inv1 row: 1/sqrt(ss1) = exp(-0.5*ln(ss1))
            inv1_row = small.tile([1, S], BF16, name="inv1_row", tag="inv1_row")
            nc.scalar.activation(
                out=inv1_row, in_=lns1,
                func=mybir.ActivationFunctionType.Exp, scale=-0.5,
            )

            # inv2/BW in column form: transpose ln(ss2) then
            # exp(-0.5*ln(ss) + ln(1/BW)) in one activation
            c2_ps = ps_small.tile([128, 4], F32, name="c2_ps", tag="sm_ps_b")
            for m in range(4):
                nc.tensor.transpose(
                    out=c2_ps[:, m : m + 1],
                    in_=lns2[0:1, 128 * m : 128 * (m + 1)],
                    identity=one_1x1,
                )
            inv2bw = small.tile([128, 4], F32, name="inv2bw", tag="inv2bw")
            nc.scalar.activation(
                out=inv2bw, in_=c2_ps,
                func=mybir.ActivationFunctionType.Exp,
                scale=-0.5, bias=ln_ibw_col,
            )

            # --- normalized t1 in bf16: f1nb = f1b * bcast(inv1) ---
            bc_ps = ps.tile([128, S], F32, name="bc_ps", tag="big_ps")
            nc.tensor.matmul(out=bc_ps, lhsT=ones_row, rhs=inv1_row)
            bc = tmp.tile([128, S], F32, name="bc", tag="bc")
            nc.any.tensor_copy(out=bc, in_=bc_ps)

            # --- G = t2_raw^T @ t1_raw (bf16), col-scale by inv1, exp with
            # inv2/BW row scale ---
            e = [big.tile([128, S], BF16, name=f"e_{m}", tag=f"e_{m}") for m in range(4)]
            for m in range(4):
                g_ps = ps.tile([128, S], F32, name="g_ps", tag="big_ps")
                msl = slice(128 * m, 128 * (m + 1))
                for i in range(2):
                    nc.tensor.matmul(
                        out=g_ps, lhsT=f2b[i][:, msl], rhs=f1b[i],
                        start=(i == 0), stop=(i == 1),
                    )
                gsc = tmp.tile([128, S], F32, name="gsc", tag="gsc")
                nc.vector.tensor_tensor(
                    out=gsc, in0=g_ps, in1=bc, op=mybir.AluOpType.mult
                )
                nc.scalar.activation(
                    out=e[m], in_=gsc,
                    func=mybir.ActivationFunctionType.Exp,
                    scale=inv2bw[:, m : m + 1],
                )

            # --- E^T via 16 PE transposes (bf16) ---
            et = [big.tile([128, S], BF16, name=f"et_{a}", tag=f"et_{a}") for a in range(4)]
            for a in range(4):
                et_ps = ps.tile([128, S], BF16, name="et_ps", tag="big_ps")
                asl = slice(128 * a, 128 * (a + 1))
                for m in range(4):
                    nc.tensor.transpose(
                        out=et_ps[:, 128 * m : 128 * (m + 1)],
                        in_=e[m][:, asl],
                        identity=ident_bf,
                    )
                nc.any.tensor_copy(out=et[a], in_=et_ps)

            # --- diag(E) in column form ---
            dcol = small.tile([128, 4], F32, name="dcol", tag="dcol")
            for m in range(4):
                scr = tmp.tile([128, 128], F32, name="scr_diag", tag="scr_diag")
                nc.vector.tensor_tensor(
                    out=scr,
                    in0=e[m][:, 128 * m : 128 * (m + 1)],
                    in1=ident_bf,
                    op=mybir.AluOpType.mult,
                )
                nc.vector.tensor_reduce(
                    out=dcol[:, m : m + 1], in_=scr,
                    axis=mybir.AxisListType.X, op=mybir.AluOpType.add,
                )

            # --- RS/CS matmuls + loss terms per 128-row block of s ---
            acc = small.tile([128, 4], F32, name="acc", tag="acc")
            for m in range(4):
                msl = slice(128 * m, 128 * (m + 1))
                rs_ps = ps_small.tile([128, PQ], F32, name="rs_ps", tag="sm_ps_a")
                cs_ps = ps_small.tile([128, PQ], F32, name="cs_ps", tag="sm_ps_b")
                for t in range(4):
                    nc.tensor.matmul(
                        out=rs_ps, lhsT=et[t][:, msl], rhs=cnt[t],
                        start=(t == 0), stop=(t == 3),
                    )
                for t in range(4):
                    nc.tensor.matmul(
                        out=cs_ps, lhsT=e[t][:, msl], rhs=cnt[t],
                        start=(t == 0), stop=(t == 3),
                    )
                rinv = small.tile([128, PQ], F32, name="rinv", tag="rinv")
                cinv = small.tile([128, PQ], F32, name="cinv", tag="cinv")
                nc.vector.reciprocal(out=rinv, in_=rs_ps)
                nc.vector.reciprocal(out=cinv, in_=cs_ps)
                ssum = small.tile([128, PQ], F32, name="ssum", tag="ssum")
                nc.vector.tensor_tensor(
                    out=ssum, in0=rinv, in1=cinv, op=mybir.AluOpType.add
                )
                x = small.tile([128, PQ], F32, name="x", tag="x")
                nc.vector.tensor_scalar_mul(out=x, in0=ssum, scalar1=dcol[:, m : m + 1])
                g = small.tile([128, PQ], F32, name="g", tag="g")
                nc.scalar.activation(
                    out=g, in_=x,
                    func=mybir.ActivationFunctionType.Ln, scale=0.5, bias=eps_col,
                )
                scr2 = small.tile([128, PQ], F32, name="scr2", tag="scr2")
                nc.vector.tensor_tensor(
                    out=scr2, in0=g, in1=cnt[m], op=mybir.AluOpType.mult
                )
                nc.vector.tensor_reduce(
                    out=acc[:, m : m + 1], in_=scr2,
                    axis=mybir.AxisListType.X, op=mybir.AluOpType.add,
                )
            nc.sync.dma_start(out=partial[:, :], in_=acc)

    nc.compile()
    return nc


_NC = None


def _run(t2_feat, t1_feat, idx, trace=False, trace_kwargs=None):
    global _NC
    if _NC is None:
        _NC = _build_program()

    t2 = np.ascontiguousarray(np.asarray(t2_feat, np.float32).reshape(B, C, S))
    t1 = np.ascontiguousarray(np.asarray(t1_feat, np.float32).reshape(B, C, S))
    idx = np.asarray(idx)

    counts = np.zeros((P, S), np.float32)
    np.add.at(counts, (np.arange(P)[:, None], idx), 1.0)

    in_maps = []
    for core in range(N_CORES):
        b, q = divmod(core, 4)
        in_maps.append(
            {
                "feat2": t2[b].astype(ml_dtypes.bfloat16),
                "feat1": t1[b].astype(ml_dtypes.bfloat16),
                "cntp": np.ascontiguousarray(
                    counts[PQ * q : PQ * (q + 1)]
                    .T.reshape(4, 128, PQ)
                    .transpose(1, 0, 2)
                    .reshape(128, 128)
                ).astype(ml_dtypes.bfloat16),
            }
        )

    kwargs = {}
    if trace:
        kwargs = dict(trace=True, trace_kwargs=trace_kwargs or {})
    res = run_bass_kernel_spmd(_NC, in_maps, core_ids=list(range(N_CORES)), **kwargs)
    total = sum(r["partial"].sum(dtype=np.float64) for r in res.results)
    loss = -total / (P * B * K)
    return np.array(loss, dtype=np.float32), res


def kernel(t2_feat, t1_feat, idx):
    out, _ = _run(t2_feat, t1_feat, idx)
    return out



# revision 35
# speedup vs baseline: 1.9142x; 1.9142x over previous
"""Contrastive patch loss (InfoNCE over sampled voxel patches) on 8 TRN2 NeuronCores.

Math
----
Reference computes, per patch p and batch b, cs[k,l] = <t2n[:,i_pk], t1n[:,i_pl]>
over k=512 sampled voxels (i = idx[p]), e = exp(cs/bw), then the mean over
(p,b,j) of -log(0.5*e_jj*(1/colsum_j + 1/rowsum_j) + eps).

Since every sampled voxel index lives in [0, 512), cs is a gather of the
512x512 Gram matrix G_b = t2n^T @ t1n.  With E_b = exp(G_b/bw) and c_p[s] the
multiplicity of voxel s in patch p:

    rowsum_j = (E_b @ c_p)[i_j]        colsum_j = (E_b^T @ c_p)[i_j]
    pos_j    = diag(E_b)[i_j]

so the whole (P,B,K,K) tensor never exists:

    loss = -1/(P*B*K) * sum_{b,p,s} c_p[s] *
           log(0.5*diagE_b[s]*(1/CS_b[s,p] + 1/RS_b[s,p]) + eps)

E is stored with a constant exponent offset (E' = E*e^-OFF, fp8e4m3): the
offset cancels in diag/RS and diag/CS, keeping the formula unchanged while
fitting e4m3 range.

Sharding: 8 cores = 2 batches x 4 patch-quarters; per-core partial sums are
returned as a (128,4) tile and summed on the host (no collectives).

Precision: Gram operands are fp8e4m3 (DoubleRow perf mode: 256-deep
contraction in a single matmul at 0.5 cycles/row); norms come from bf16
squares; E/E^T tiles are fp8 (exponent-offset); accumulation, exp/log and the
loss reduction stay fp32.
"""

import math
import os

import ml_dtypes
import numpy as np

import concourse.bacc as bacc
import concourse.tile as tile
from concourse import hw_specs, mybir
from concourse.bass_utils import run_bass_kernel_spmd

# Pin every ACTIVATE to the one table set that holds ln+exp+square+copy, so
# the kernel pays a single ACT_TABLE_LOAD instead of ping-ponging between the
# per-function default sets.
_PIN_SET = "natural_log_exp_and_others"
_orig_get_tables = hw_specs.get_activation_tables


def _pinned_tables(arch):
    tabs = _orig_get_tables(arch)
    return {k: (v if k == _PIN_SET else set()) for k, v in tabs.items()}


bacc.get_activation_tables = _pinned_tables

B, C, S = 2, 256, 512
P, K = 128, 512
BW = 0.05
EPS = 1e-5
N_CORES = 8
PQ = P // 4  # patches per core (32)
EOFF = 1.5  # exponent offset: E' = exp(cs/bw - EOFF); cancels in pos/sum
F32 = mybir.dt.float32
BF16 = mybir.dt.bfloat16
FP8 = mybir.dt.float8e4
DR = mybir.MatmulPerfMode.DoubleRow

USE_DR = os.environ.get("K_NO_DR") != "1"  # DoubleRow fp8 Gram
USE_E8 = os.environ.get("K_NO_E8") != "1"  # fp8 E/E^T tiles
USE_TTR = os.environ.get("K_NO_TTR") != "1"  # fused tensor_tensor_reduce
STAGE = int(os.environ.get("K_STAGE", "6"))
DEBUG_DUMP = os.environ.get("K_DEBUG") == "1"
EDT = FP8 if USE_E8 else BF16


def _build_program():
    nc = bacc.Bacc("TRN2", target_bir_lowering=False, debug=False, num_devices=N_CORES)

    # fp8 features, channel-interleaved for DoubleRow: fx[p,i,s] = f[i*128+p, s]
    fx8 = nc.dram_tensor("fx8", [128, 2, S], FP8, kind="ExternalInput")
    fy8 = nc.dram_tensor("fy8", [128, 2, S], FP8, kind="ExternalInput")
    # bf16 raw features (for norms), same layout
    fxu = nc.dram_tensor("fxu", [128, 2, S], BF16, kind="ExternalInput")
    fyu = nc.dram_tensor("fyu", [128, 2, S], BF16, kind="ExternalInput")
    # counts C^T blocks: cnt[p, 32t+j] = counts[32q+j, 128t+p]
    cntp = nc.dram_tensor("cntp", [128, 128], BF16, kind="ExternalInput")
    identd = nc.dram_tensor("identd", [128, 128], BF16, kind="ExternalInput")
    identd8 = nc.dram_tensor("identd8", [128, 128], FP8, kind="ExternalInput")
    identdh = nc.dram_tensor("identdh", [128, 128], BF16, kind="ExternalInput")
    partial = nc.dram_tensor("partial", [128, 4], F32, kind="ExternalOutput")
    if DEBUG_DUMP:
        dbg_e = nc.dram_tensor("dbg_e", [128, 4, S], FP8, kind="ExternalOutput")
        dbg_etm = nc.dram_tensor("dbg_etm", [128, 4, S], FP8, kind="ExternalOutput")
        dbg_rinv = nc.dram_tensor("dbg_rinv", [128, 128], F32, kind="ExternalOutput")
        dbg_cinv = nc.dram_tensor("dbg_cinv", [128, 128], F32, kind="ExternalOutput")
        dbg_dcol = nc.dram_tensor("dbg_dcol", [128, 4], F32, kind="ExternalOutput")
        dbg_inv2 = nc.dram_tensor("dbg_inv2", [128, 4], F32, kind="ExternalOutput")
        dbg_bc = nc.dram_tensor("dbg_bc", [128, S], BF16, kind="ExternalOutput")

    with tile.TileContext(nc) as tc:
        with (
            tc.tile_pool(name="const", bufs=1) as const,
            tc.tile_pool(name="feat", bufs=1) as featp,
            tc.tile_pool(name="big", bufs=1) as big,
            tc.tile_pool(name="tmp", bufs=2) as tmp,
            tc.tile_pool(name="small", bufs=2) as small,
            tc.tile_pool(name="ps_g", bufs=2, space="PSUM") as ps_g,
            tc.tile_pool(name="ps_t", bufs=2, space="PSUM") as ps_t,
            tc.tile_pool(name="ps_cs", bufs=1, space="PSUM") as ps_cs,
            tc.tile_pool(name="ps_misc", bufs=1, space="PSUM") as ps_misc,
        ):
            # ---- input DMAs first: land while the engines boot ----
            fx = featp.tile([128, 2, S], FP8, name="fx", tag="fx")
            fy = featp.tile([128, 2, S], FP8, name="fy", tag="fy")
            f2u = featp.tile([128, 2, S], BF16, name="f2u", tag="f2u")
            f1u = featp.tile([128, 2, S], BF16, name="f1u", tag="f1u")
            cnt_all = const.tile([128, 128], BF16, name="cnt_all", tag="cnt_all")
            ident = const.tile([128, 128], BF16, name="ident", tag="ident")
            ident8 = const.tile([128, 128], FP8, name="ident8", tag="ident8")
            nc.sync.dma_start(out=fx, in_=fx8[:, :, :])
            nc.sync.dma_start(out=fy, in_=fy8[:, :, :])
            nc.sync.dma_start(out=f1u, in_=fyu[:, :, :])
            nc.sync.dma_start(out=f2u, in_=fxu[:, :, :])
            nc.sync.dma_start(out=cnt_all, in_=cntp[:, :])
            nc.sync.dma_start(out=ident, in_=identd[:, :])
            nc.sync.dma_start(out=ident8, in_=identd8[:, :])
            identh = const.tile([128, 128], BF16, name="identh", tag="identh")
            nc.sync.dma_start(out=identh, in_=identdh[:, :])
            cnt = [cnt_all[:, PQ * t : PQ * (t + 1)] for t in range(4)]

            ones_col = const.tile([128, 1], BF16, name="ones_col", tag="ocb")
            nc.vector.memset(ones_col, 1.0)
            ones_row = const.tile([1, 128], BF16, name="ones_row", tag="ones_row")
            nc.vector.memset(ones_row, 1.0)
            eps_col = const.tile([128, 1], F32, name="eps_col", tag="eps_col")
            nc.vector.memset(eps_col, EPS)
            ln_ibw_col = const.tile([128, 1], F32, name="ln_ibw_col", tag="lbc")
            nc.vector.memset(ln_ibw_col, math.log(1.0 / BW))
            off_col = const.tile([128, 1], F32, name="off_col", tag="off_col")
            nc.vector.memset(off_col, -EOFF)

            acc = small.tile([128, 4], F32, name="acc", tag="acc")
            if STAGE < 6:
                nc.vector.memset(acc, 0.0)

            if STAGE >= 2:
                # ---- squares (DVE, bf16 fast path) ----
                sq1 = tmp.tile([128, 2, S], BF16, name="sq1", tag="sq1")
                nc.vector.tensor_tensor(
                    out=sq1, in0=f1u, in1=f1u, op=mybir.AluOpType.mult
                )
                sq2 = tmp.tile([128, 2, S], BF16, name="sq2", tag="sq2")
                nc.vector.tensor_tensor(
                    out=sq2, in0=f2u, in1=f2u, op=mybir.AluOpType.mult
                )

                # ---- PE: ss1 row, ss2 cols ----
                ss1_ps = ps_misc.tile([1, S], F32, name="ss1_ps", tag="ss1_ps")
                for i in range(2):
                    nc.tensor.matmul(
                        out=ss1_ps, lhsT=ones_col, rhs=sq1[:, i, :],
                        start=(i == 0), stop=(i == 1),
                    )
                # one PSUM bank: rs (cols 0:128), cs (128:256), ss2 (256:260)
                mega_ps = ps_misc.tile([128, 260], F32, name="mega_ps", tag="mega_ps")
                ss2c_ps = mega_ps[:, 256:260]
                rs_ps = mega_ps[:, 0:128]
                cs_ps = mega_ps[:, 128:256]
                for m in range(4):
                    msl = slice(128 * m, 128 * (m + 1))
                    for i in range(2):
                        nc.tensor.matmul(
                            out=ss2c_ps[:, m : m + 1], lhsT=sq2[:, i, msl],
                            rhs=ones_col, start=(i == 0), stop=(i == 1),
                        )

            if STAGE >= 3:
                # Gram: one DoubleRow matmul per 128-row block (contraction 256)
                g_ps = []
                for m in range(4):
                    gp = ps_g.tile([128, S], F32, name=f"g_ps{m}", tag="g_ps")
                    if USE_DR:
                        nc.tensor.matmul(
                            out=gp, lhsT=fx[:, :, 128 * m : 128 * (m + 1)], rhs=fy,
                            perf_mode=DR, start=True, stop=True,
                        )
                    else:
                        for i in range(2):
                            nc.tensor.matmul(
                                out=gp, lhsT=fx[:, i, 128 * m : 128 * (m + 1)],
                                rhs=fy[:, i, :], start=(i == 0), stop=(i == 1),
                            )
                    g_ps.append(gp)

            if STAGE >= 2:
                # ---- ACT: norms ----
                lns1 = small.tile([1, S], F32, name="lns1", tag="lns1")
                nc.scalar.activation(
                    out=lns1, in_=ss1_ps, func=mybir.ActivationFunctionType.Ln
                )
                inv1_row = small.tile([1, S], BF16, name="inv1_row", tag="inv1_row")
                nc.scalar.activation(
                    out=inv1_row, in_=lns1,
                    func=mybir.ActivationFunctionType.Exp, scale=-0.5,
                )
                lnc2 = small.tile([128, 4], F32, name="lnc2", tag="lnc2")
                nc.scalar.activation(
                    out=lnc2, in_=ss2c_ps, func=mybir.ActivationFunctionType.Ln
                )
                inv2bw = small.tile([128, 4], F32, name="inv2bw", tag="inv2bw")
                nc.scalar.activation(
                    out=inv2bw, in_=lnc2,
                    func=mybir.ActivationFunctionType.Exp,
                    scale=-0.5, bias=ln_ibw_col,
                )

                # bc[p, s] = inv1[s] broadcast (PE outer product with ones)
                bc_ps = ps_misc.tile([128, S], F32, name="bc_ps", tag="bc_ps")
                nc.tensor.matmul(out=bc_ps, lhsT=ones_row, rhs=inv1_row)
                bc = big.tile([128, S], BF16, name="bc", tag="bc")
                nc.vector.tensor_copy(out=bc, in_=bc_ps)

            if STAGE >= 3:
                # ---- per-tile: col-scale (DVE), exp (ACT) ----
                e = [
                    big.tile([128, S], EDT, name=f"e_{m}", tag=f"e_{m}")
                    for m in range(4)
                ]
                for m in range(4):
                    g = tmp.tile([128, S], F32, name=f"gsc{m}", tag="gsc")
                    nc.vector.tensor_tensor(
                        out=g, in0=g_ps[m], in1=bc, op=mybir.AluOpType.mult
                    )
                    nc.scalar.activation(
                        out=e[m], in_=g,
                        func=mybir.ActivationFunctionType.Exp,
                        scale=inv2bw[:, m : m + 1], bias=off_col,
                    )

            if STAGE >= 4:
                etm = [
                    big.tile([128, S], EDT, name=f"etm_{m}", tag=f"etm_{m}")
                    for m in range(4)
                ]
                dcol = small.tile([128, 4], F32, name="dcol", tag="dcol")

            def emit_transposes(m):
                # fp8 transpose writes PSUM with element step 2: stage in a
                # [128, S, 2] tile and use the stride-2 view as the output.
                if USE_E8:
                    et_full = ps_t.tile([128, S, 2], FP8, name=f"et_ps{m}", tag="et_ps")
                    et_ps = et_full[:, :, 0]
                    idn = ident8
                else:
                    et_full = ps_t.tile([128, S], BF16, name=f"et_ps{m}", tag="et_ps")
                    et_ps = et_full[:, :]
                    idn = ident
                for a in range(4):
                    nc.tensor.transpose(
                        out=et_ps[:, 128 * a : 128 * (a + 1)],
                        in_=e[m][:, 128 * a : 128 * (a + 1)],
                        identity=idn,
                    )
                # PSUM->SBUF move; GpSimd has no PSUM access, so alternate
                # the two engines that do and are least loaded here.
                if m % 2 == 0:
                    nc.scalar.activation(
                        out=etm[m], in_=et_ps,
                        func=mybir.ActivationFunctionType.Copy,
                    )
                else:
                    nc.vector.tensor_copy(out=etm[m], in_=et_ps)

            # PSUM accumulation groups must be consecutive matmuls on hw, and
            # the CS t-passes interleave with transposes/RS.  So each t-pass
            # accumulates into its own fresh PSUM tile (4 back-to-back mms),
            # and the cross-t sum is done on DVE into SBUF.
            cs_acc = small.tile([128, 128], F32, name="cs_acc", tag="cs_acc")

            def emit_cs_pass(t):
                ctp = ps_cs.tile([128, 128], F32, name=f"cs_t{t}", tag="cs_t")
                for mp in range(4):
                    nc.tensor.matmul(
                        out=ctp[:, PQ * mp : PQ * (mp + 1)],
                        lhsT=e[t][:, 128 * mp : 128 * (mp + 1)],
                        rhs=cnt[t], start=True, stop=True,
                    )
                if t == 0:
                    nc.vector.tensor_copy(out=cs_acc, in_=ctp)
                else:
                    nc.vector.tensor_tensor(
                        out=cs_acc, in0=cs_acc, in1=ctp, op=mybir.AluOpType.add
                    )

            def emit_rs(m):
                for a in range(4):
                    nc.tensor.matmul(
                        out=rs_ps[:, PQ * m : PQ * (m + 1)],
                        lhsT=etm[m][:, 128 * a : 128 * (a + 1)],
                        rhs=cnt[a], start=(a == 0), stop=(a == 3),
                    )

            def emit_dcol(m):
                scr = tmp.tile([128, 128], BF16, name=f"scr{m}", tag="scr")
                if USE_TTR:
                    nc.vector.tensor_tensor_reduce(
                        out=scr, in0=e[m][:, 128 * m : 128 * (m + 1)], in1=ident,
                        scale=0.5, scalar=0.0,
                        op0=mybir.AluOpType.mult, op1=mybir.AluOpType.add,
                        accum_out=dcol[:, m : m + 1],
                    )
                else:
                    nc.vector.tensor_tensor(
                        out=scr, in0=e[m][:, 128 * m : 128 * (m + 1)], in1=identh,
                        op=mybir.AluOpType.mult,
                    )
                    nc.vector.tensor_reduce(
                        out=dcol[:, m : m + 1], in_=scr,
                        axis=mybir.AxisListType.X, op=mybir.AluOpType.add,
                    )

            if STAGE >= 4:
                # PE order tuned so RS(m) lands after the etm copy is done.
                emit_transposes(0)
                if STAGE >= 5:
                    emit_cs_pass(0)
                emit_dcol(0)
                emit_transposes(1)
                if STAGE >= 5:
                    emit_cs_pass(1)
                emit_dcol(1)
                if STAGE >= 5:
                    emit_rs(0)
                emit_transposes(2)
                if STAGE >= 5:
                    emit_cs_pass(2)
                emit_dcol(2)
                if STAGE >= 5:
                    emit_rs(1)
                emit_transposes(3)
                if STAGE >= 5:
                    emit_cs_pass(3)
                emit_dcol(3)
                if STAGE >= 5:
                    emit_rs(2)
                    emit_rs(3)

            if STAGE >= 6:
                # ---- tail: per-m  ln(0.5*d*(1/RS+1/CS) + eps) dot cnt ----
                rinv = small.tile([128, 128], F32, name="rinv", tag="rinv")
                cinv = small.tile([128, 128], F32, name="cinv", tag="cinv")
                for m in range(4):
                    mcol = slice(PQ * m, PQ * (m + 1))
                    nc.vector.reciprocal(out=rinv[:, mcol], in_=rs_ps[:, mcol])
                    nc.vector.reciprocal(out=cinv[:, mcol], in_=cs_acc[:, mcol])
                    ssum = small.tile([128, PQ], F32, name=f"ssum{m}", tag="ssum")
                    nc.vector.tensor_tensor(
                        out=ssum, in0=rinv[:, mcol], in1=cinv[:, mcol],
                        op=mybir.AluOpType.add,
                    )
                    g = small.tile([128, PQ], F32, name=f"gl{m}", tag="gl")
                    nc.scalar.activation(
                        out=g, in_=ssum,
                        func=mybir.ActivationFunctionType.Ln,
                        scale=dcol[:, m : m + 1], bias=eps_col,
                    )
                    scr2 = small.tile([128, PQ], F32, name=f"scr2_{m}", tag="scr2")
                    if USE_TTR:
                        nc.vector.tensor_tensor_reduce(
                            out=scr2, in0=g, in1=cnt[m],
                            scale=1.0, scalar=0.0,
                            op0=mybir.AluOpType.mult, op1=mybir.AluOpType.add,
                            accum_out=acc[:, m : m + 1],
                        )
                    else:
                        nc.vector.tensor_tensor(
                            out=scr2, in0=g, in1=cnt[m], op=mybir.AluOpType.mult
                        )
                        nc.vector.tensor_reduce(
                            out=acc[:, m : m + 1], in_=scr2,
                            axis=mybir.AxisListType.X, op=mybir.AluOpType.add,
                        )
            nc.sync.dma_start(out=partial[:, :], in_=acc)
            if DEBUG_DUMP:
                for m in range(4):
                    nc.sync.dma_start(out=dbg_e[:, m, :], in_=e[m])
                    nc.sync.dma_start(out=dbg_etm[:, m, :], in_=etm[m])
                nc.sync.dma_start(out=dbg_rinv[:, :], in_=rinv)
                nc.sync.dma_start(out=dbg_cinv[:, :], in_=cinv)
                nc.sync.dma_start(out=dbg_dcol[:, :], in_=dcol)
                nc.sync.dma_start(out=dbg_inv2[:, :], in_=inv2bw)
                nc.sync.dma_start(out=dbg_bc[:, :], in_=bc)

    nc.compile()
    return nc


_NC = None


def _pack_inputs(t2, t1, idx):
    counts = np.zeros((P, S), np.float32)
    np.add.at(counts, (np.arange(P)[:, None], idx), 1.0)
    ident = np.eye(128, dtype=ml_dtypes.bfloat16)

    in_maps = []
    for core in range(N_CORES):
        b, q = divmod(core, 4)
        f2i = np.ascontiguousarray(t2[b].reshape(2, 128, S).transpose(1, 0, 2))
        f1i = np.ascontiguousarray(t1[b].reshape(2, 128, S).transpose(1, 0, 2))
        cq = np.ascontiguousarray(
            counts[PQ * q : PQ * (q + 1)]
            .T.reshape(4, 128, PQ)
            .transpose(1, 0, 2)
            .reshape(128, 128)
        )
        in_maps.append(
            {
                "fx8": f2i.astype(ml_dtypes.float8_e4m3fn),
                "fy8": f1i.astype(ml_dtypes.float8_e4m3fn),
                "fxu": f2i.astype(ml_dtypes.bfloat16),
                "fyu": f1i.astype(ml_dtypes.bfloat16),
                "cntp": cq.astype(ml_dtypes.bfloat16),
                "identd": ident,
                "identd8": ident.astype(ml_dtypes.float8_e4m3fn),
                "identdh": (0.5 * ident.astype(np.float32)).astype(
                    ml_dtypes.bfloat16
                ),
            }
        )
    return in_maps


def _run(t2_feat, t1_feat, idx, trace=False, trace_kwargs=None):
    global _NC
    if _NC is None:
        _NC = _build_program()

    t2 = np.ascontiguousarray(np.asarray(t2_feat, np.float32).reshape(B, C, S))
    t1 = np.ascontiguousarray(np.asarray(t1_feat, np.float32).reshape(B, C, S))
    idx = np.asarray(idx)
    in_maps = _pack_inputs(t2, t1, idx)

    kwargs = {}
    if trace:
        kwargs = dict(trace=True, trace_kwargs=trace_kwargs or {})
    res = run_bass_kernel_spmd(_NC, in_maps, core_ids=list(range(N_CORES)), **kwargs)
    total = sum(r["partial"].sum(dtype=np.float64) for r in res.results)
    loss = -total / (P * B * K)
    return np.array(loss, dtype=np.float32), res


def kernel(t2_feat, t1_feat, idx):
    out, _ = _run(t2_feat, t1_feat, idx)
    return out
